# revision 16
# baseline (speedup 1.0000x reference)
"""AWQ int4 linear + fused LoRA on 8 Trainium2 NeuronCores.

Computes out = x @ dequant(qweight, qzeros, scales) + (x @ lora_a) @ lora_b
with tensor-parallel column sharding over N (no collectives needed).

Per-core device kernel:
  Phase A: dequantize the per-core weight shard W = (q - z) * s into SBUF
    (bf16), folding in the LoRA product A@B.  q ships as uint8 [K, NP].
    Per (group j, n-slice): one PE matmul broadcasts s_j across partitions
    (indicator lhsT), one PE matmul computes the combined correction
    (A@B - z*s, with -z*s split hi+lo bf16 for precision) from a single
    resident [128, K] lhsT = [A^T; ind; ind] against a single resident
    [128, NP] rhs = [B; c_hi; c_lo], and two DVE ops fuse
    W = q * s_bcast + corr with one bf16 rounding.
  Phase B: dense bf16 GEMM against the resident W, fp32 PSUM accumulate.
    x arrives host-pre-cast to bf16 in a supertile-contiguous layout
    [P, S, NG, TSUP] so each x-tile DMA is 128 x 16KB contiguous lines.
    Outputs evict PSUM -> bf16 SBUF -> DRAM (host casts back to fp32).
"""

import sys

if "/opt/trn_rl_repo" not in sys.path:
    sys.path.insert(0, "/opt/trn_rl_repo")

import numpy as np

P = 128
N_CORES = 8
T_FULL, K_FULL, N_FULL = 8192, 4096, 11008
R_FULL = 64
NSH = N_FULL // N_CORES  # 1376 columns per core
NP_FULL = NSH


def _n_slices(NP, max_free=512):
    out = []
    off = 0
    while off < NP:
        ns = min(max_free, NP - off)
        out.append((off, ns))
        off += ns
    return out


def _patched_tile_context(tile_mod, nc):
    """TileContext whose tail drain keeps <=1 sem wait per SP instruction
    (this walrus build rejects >2 sync waits on a Drain)."""
    from bass_rust import ScopedClock, SyncInfo

    class TileContextPatched(tile_mod.TileContext):
        def _drain_and_barrier(self, tick_clock, wait_clock):
            drain_inst = self.nc.sync.drain()
            wait_clock.add_sem_waits(
                drain_inst.ins, ScopedClock({None: tick_clock.global_clock})
            )
            waits = list(drain_inst.ins.sync_info.on_wait)
            if len(waits) > 1:
                drain_inst.ins.sync_info.on_wait = waits[:1]
                for w in waits[1:]:
                    nop = self.nc.sync.nop()
                    nop.ins.sync_info = SyncInfo(on_wait=[w], on_update=[])

            self.nc.all_engine_barrier()
            assert self.sems is not None
            popped = self.nc._tile_sem_poison_stack.pop()
            assert popped is self._sem_poison
            self.nc.clear_and_free_semaphores(list(self.sems.allocated().values()))
            self.nc.all_engine_barrier()

    return TileContextPatched(nc)


def _split_multi_waits(nc, max_waits=1):
    """This walrus build rejects instructions carrying more than ~1-2 sem
    waits ('Too many sync wait commands').  Move extra waits onto standalone
    EventSemaphore instructions inserted just before, on the same engine —
    engines execute their stream in order, so this is semantically identical.
    """
    from concourse import mybir

    n_split = 0
    for f in nc.m.functions:
        for bb in f.blocks:
            insts = list(bb.instructions)
            out, changed = [], False
            for inst in insts:
                si = inst.sync_info
                if si is not None and len(si.on_wait) > max_waits:
                    waits = list(si.on_wait)
                    for w in waits[:-max_waits]:
                        n_split += 1
                        nop = mybir.InstEventSemaphore(
                            name=f"{inst.name}-ws{n_split}", ins=[], outs=[])
                        nop.engine = inst.engine
                        nop.sync_info = mybir.SyncInfo(on_wait=[w], on_update=[])
                        out.append(nop)
                    si.on_wait = waits[-max_waits:]
                    changed = True
                out.append(inst)
            if changed:
                bb.instructions = out
    return n_split


ALL_FEATURES = frozenset({"phase_a", "xload", "mm", "evict", "store"})


def build_bass(T=T_FULL, K=K_FULL, NP=NP_FULL, R=R_FULL, TSUP=256,
               num_devices=N_CORES, split_waits=True, repeat=1,
               features=ALL_FEATURES, xb_bufs=4, deq_bufs=4,
               pa_ps_bufs=1, mm_ps_bufs=6):
    """Build the per-core Bass program (SPMD: all cores run this)."""
    import concourse.bass as bass
    import concourse.tile as tile
    from concourse import mybir

    NG = K // P  # k-tiles; == quant groups (group size 128)
    S = T // TSUP
    assert T % TSUP == 0 and TSUP % P == 0
    f32, bf16 = mybir.dt.float32, mybir.dt.bfloat16
    u8 = mybir.dt.uint8

    nc = bass.Bass("TRN2", target_bir_lowering=False, debug=False,
                   num_devices=num_devices)
    # x pre-cast bf16, supertile-contiguous: [P, S, NG, TSUP]
    xbf_d = nc.dram_tensor("xbf", [P, S, NG, TSUP], bf16, kind="ExternalInput")
    q_d = nc.dram_tensor("q", [K, NP], u8, kind="ExternalInput")
    # per-partition-packed scales/zeros: [NG, 2*NP] f32 = [s | z]
    sz_d = nc.dram_tensor("sz", [NG, 2 * NP], f32, kind="ExternalInput")
    # combined correction lhsT rows: [A^T (R); ind (NG); ind (NG)] bf16
    clhs_d = nc.dram_tensor("clhs", [R + 2 * NG, K], bf16, kind="ExternalInput")
    # correction rhs [128, NP] bf16: rows 0..R-1 = B, rest zero (filled on dev)
    crhs_d = nc.dram_tensor("crhs", [R + 2 * NG, NP], bf16,
                            kind="ExternalInput")
    indic_d = nc.dram_tensor("indic", [NG, K], f32, kind="ExternalInput")
    out_d = nc.dram_tensor("out", [T, NP], bf16, kind="ExternalOutput")

    slices = _n_slices(NP)
    CL = R + 2 * NG  # 128 correction-lhsT rows

    from contextlib import ExitStack

    tc = _patched_tile_context(tile, nc)
    with tc, ExitStack() as ctx:
        f32r = mybir.dt.float32r
        const = ctx.enter_context(tc.tile_pool(name="const", bufs=1))
        # indicator resident [NG, K] f32r for the s-broadcast matmuls
        ind_sb = const.tile([NG, K], f32r, name="ind_sb")
        nc.gpsimd.dma_start(ind_sb[:], indic_d.ap())
        # scales+zeros resident [NG, 2*NP] f32r (s | z on same partitions)
        sz_sb = const.tile([NG, 2 * NP], f32r, name="sz_sb")
        nc.gpsimd.dma_start(sz_sb[:], sz_d.ap())
        s_sb = sz_sb[:, 0:NP]
        # correction lhsT resident [128, K] bf16
        clhs_sb = const.tile([CL, K], bf16, name="clhs_sb")
        nc.gpsimd.dma_start(clhs_sb[:], clhs_d.ap())
        # correction rhs resident [128, NP] bf16: [B; c_hi; c_lo]
        crhs_sb = const.tile([CL, NP], bf16, name="crhs_sb")
        nc.gpsimd.dma_start(crhs_sb[:], crhs_d.ap())
        with tc.tile_pool(name="zinit", bufs=1) as zinit:
            negc = zinit.tile([NG, NP], f32, name="negc")
            nc.vector.tensor_mul(negc[:], sz_sb[:, NP:2 * NP].bitcast(f32),
                                 s_sb.bitcast(f32))
            nc.vector.tensor_scalar(out=negc[:], in0=negc[:], scalar1=-1.0,
                                    scalar2=None, op0=mybir.AluOpType.mult)
            # c_hi = bf16(-z*s); c_lo = bf16(-z*s - c_hi).  Computed at
            # partitions 0..31 (DVE in/out must share partitions), then
            # DMA'd into the correction-rhs rows R.. (DMA moves partitions).
            c_hi = zinit.tile([NG, NP], bf16, name="c_hi")
            c_lo = zinit.tile([NG, NP], bf16, name="c_lo")
            nc.vector.tensor_copy(c_hi[:], negc[:])
            nc.vector.tensor_sub(c_lo[:], negc[:], c_hi[:])
            nc.sync.dma_start(crhs_sb[R:R + NG, :], c_hi[:])
            nc.sync.dma_start(crhs_sb[R + NG:R + 2 * NG, :], c_lo[:])

        # one W tile per n-slice: phase B pass si depends only on slice si's
        # dequant (tile-granular deps), so the GEMM starts after 1/3 of
        # phase A instead of all of it.
        wpool = ctx.enter_context(tc.tile_pool(name="wpool", bufs=1))
        W_slices = [wpool.tile([P, NG, ns], bf16, name=f"W_sb{si}")
                    for si, (off, ns) in enumerate(slices)]
        if "phase_a" not in features:
            for W_si in W_slices:
                nc.vector.memset(W_si[:, 0:1, 0:1], 0.0)

        # All working pools coexist at one scope: phase A and phase B tiles
        # never alias addresses, so the scheduler can overlap the phases.
        deq = ctx.enter_context(tc.tile_pool(name="deq", bufs=deq_bufs))
        ps_pool = ctx.enter_context(tc.tile_pool(name="ps", bufs=1,
                                                 space="PSUM"))
        xb = ctx.enter_context(tc.tile_pool(name="xb", bufs=xb_bufs))
        ob = ctx.enter_context(tc.tile_pool(name="ob", bufs=2))

        for rep in range(repeat):
          # ---- Phase A: dequant + LoRA fold (slice-major) ----
          if "phase_a" in features:
                for si, (off, ns) in enumerate(slices):
                    W_si = W_slices[si]
                    for j in range(NG):
                        q_t = deq.tile([P, 512], u8, name="q_t")
                        nc.sync.dma_start(
                            q_t[:, :ns],
                            q_d.ap()[j * P:(j + 1) * P, off:off + ns])
                        # broadcast s_j across partitions: psum[p,n] = s[j,n]
                        ps_s = ps_pool.tile([P, 512], f32, name="ps_s",
                                            bufs=pa_ps_bufs)
                        nc.tensor.matmul(
                            ps_s[:, :ns],
                            lhsT=ind_sb[:, j * P:(j + 1) * P],
                            rhs=s_sb[:, off:off + ns],
                            start=True, stop=True,
                        )
                        # correction chunk: (A@B - z*s) over this slice
                        ps_ab = ps_pool.tile([P, 512], f32, name="ps_ab",
                                             bufs=pa_ps_bufs)
                        nc.tensor.matmul(
                            ps_ab[:, :ns],
                            lhsT=clhs_sb[:, j * P:(j + 1) * P],
                            rhs=crhs_sb[:, off:off + ns],
                            start=True, stop=True,
                        )
                        # tmp = q * s_bcast in fp32, then single bf16
                        # rounding in the final add
                        qs_t = deq.tile([P, 512], f32, name="qs_t")
                        nc.vector.tensor_mul(qs_t[:, :ns],
                                             q_t[:, :ns],
                                             ps_s[:, :ns])
                        nc.vector.tensor_add(W_si[:, j, :],
                                             qs_t[:, :ns],
                                             ps_ab[:, :ns])

          # ---- Phase B: main GEMM in two passes over N ----
          # pass 0 = slice 0 only: it starts once 1/3 of the dequant is
          # done; pass 1 = remaining slices, by which time dequant finished.
          # x re-streams twice, alternating the two fast DGE queues.
          if True:
            for pslices in (slices[:1], slices[1:]):
                p_off = pslices[0][0]
                p_ns = sum(ns for (_, ns) in pslices)
                for sidx in range(S):
                    x_t = xb.tile([P, NG, TSUP], bf16, name="x_t")
                    if "xload" in features:
                        eng = nc.gpsimd if sidx % 2 == 0 else nc.sync
                        eng.dma_start(x_t[:], xbf_d.ap()[:, sidx])
                    else:
                        nc.vector.memset(x_t[:, 0:1, 0:1], 0.0)
                    for tsub in range(TSUP // P):
                        t0 = sidx * TSUP + tsub * P
                        psums = [ps_pool.tile([P, 512], f32, name="mm_ps",
                                              bufs=mm_ps_bufs)
                                 for _ in pslices]
                        if "mm" in features:
                            for j in range(NG):
                                lhsT = x_t[:, j][:, tsub * P:(tsub + 1) * P]
                                for pt, (off, ns) in zip(psums, pslices):
                                    si = slices.index((off, ns))
                                    nc.tensor.matmul(
                                        pt[:, :ns],
                                        lhsT=lhsT,
                                        rhs=W_slices[si][:, j, :],
                                        start=(j == 0),
                                        stop=(j == NG - 1),
                                    )
                        out_t = ob.tile([P, 896], bf16, name="out_t")
                        if "evict" in features and "mm" in features:
                            o = 0
                            for pt, (off, ns) in zip(psums, pslices):
                                nc.vector.tensor_copy(out_t[:, o:o + ns],
                                                      pt[:, :ns])
                                o += ns
                        else:
                            nc.vector.memset(out_t[:, 0:1], 0.0)
                        if "store" in features:
                            nc.sync.dma_start(
                                out_d.ap()[t0:t0 + P, p_off:p_off + p_ns],
                                out_t[:, :p_ns],
                            )
    if split_waits:
        _split_multi_waits(nc)
    return nc


def _marshal_inputs(x, scales, lora_a, lora_b, qweight, qzeros,
                    n_cores=N_CORES, NP=NP_FULL, TSUP=256):
    """Host-side sharding + layout/dtype prep (pure data movement)."""
    import ml_dtypes

    bf16 = ml_dtypes.bfloat16
    x = np.asarray(x, dtype=np.float32)
    scales = np.asarray(scales, dtype=np.float32)
    lora_a = np.asarray(lora_a, dtype=np.float32)
    lora_b = np.asarray(lora_b, dtype=np.float32)
    qweight = np.asarray(qweight, dtype=np.int32)
    qzeros = np.asarray(qzeros, dtype=np.int32)

    T, K = x.shape
    _, N = qweight.shape
    NG = scales.shape[0]
    R = lora_a.shape[1]
    S = T // TSUP
    nsh = N // n_cores

    # x -> bf16, [P, S, NG, TSUP]: k = j*P + p, t = s*TSUP + u
    xbf = np.ascontiguousarray(
        x.T.reshape(NG, P, S, TSUP).transpose(1, 2, 0, 3)
    ).astype(bf16)
    indic = np.kron(np.eye(NG, dtype=np.float32),
                    np.ones((1, P), np.float32))  # [NG, K]
    # correction lhsT: [A^T; ind; ind] bf16 (ind is 0/1: exact in bf16)
    clhs = np.concatenate([lora_a.T, indic, indic], axis=0).astype(bf16)

    in_maps = []
    for c in range(n_cores):
        lo, hi = c * nsh, (c + 1) * nsh
        q = np.ascontiguousarray(qweight[:, lo:hi]).astype(np.uint8)
        z = np.ascontiguousarray(qzeros[:, lo:hi]).astype(np.float32)
        s = np.ascontiguousarray(scales[:, lo:hi])
        # [s | z] packed along the free dim so both share partitions
        sz = np.ascontiguousarray(np.concatenate([s, z], axis=1))
        b = np.ascontiguousarray(lora_b[:, lo:hi]).astype(bf16)
        # correction rhs: rows 0..R-1 = B, rows R.. filled on device
        crhs = np.zeros((R + 2 * NG, nsh), dtype=bf16)
        crhs[:R] = b
        in_maps.append({"xbf": xbf, "q": q, "sz": sz, "clhs": clhs,
                        "crhs": crhs, "indic": indic})
    return in_maps, nsh


_NC_CACHE = {}


def kernel(x, scales, lora_a, lora_b, qweight, qzeros):
    from concourse.bass_utils import run_bass_kernel_spmd

    in_maps, nsh = _marshal_inputs(x, scales, lora_a, lora_b, qweight, qzeros)
    key = "full"
    if key not in _NC_CACHE:
        _NC_CACHE[key] = build_bass()
    nc = _NC_CACHE[key]
    res = run_bass_kernel_spmd(nc, in_maps, core_ids=list(range(N_CORES)),
                               trace=False)
    outs = [res.results[c]["out"] for c in range(N_CORES)]
    return np.concatenate(outs, axis=1).astype(np.float32)
